# revision 43
# baseline (speedup 1.0000x reference)
"""Trainium2 Bass kernel for the differentiable-JPEG layer.

Zigzag separable-DCT design (per core; data parallel over batch, 8 imgs/core):

Every matmul makes the IMAGE DATA the stationary operand and streams a small
block-diagonal DCT matrix as the moving operand.  Because PE computes
out = lhsT.T @ rhs, each stage flips the partition/free orientation of the
data -- so the blockify / transpose required between the two separable DCT
axes falls out for free and no explicit transpose or gather ever happens.
Both color conversions are folded into the PE stages as per-(outch,inch)
scaled variants of the moving DCT matrix, reusing each stationary data
slice three times.

All row/col spaces are chunked 112+112 (14 blocks of 8), and the two chunks
of every intermediate live side by side in one [112, 448] tile, so each
elementwise op covers a full (img, ch) plane in one instruction.

Per (img, ch) with X = [rows 224 = (bi,r), cols 224 = (bj,c)]:
  Z1: VT[ycc] [p=w-chunk, f=(m,(bi,i))] += X-slice.T @ (colw * blockdiag(H))
  (plain ACT evacuation to SBUF + 2 DMA'd bias K-rows)
  Z2: C[ycc]  [p=(bi,i)-chunk, f=(m,(bj,j))] = Yt-slice.T @ A2[ch-variant]
      (A2 carries two bias rows: DC spike at (i=0,j=0) and uniform -1/2)
  quant: rec = round(t5) + sigmoid(2p*(t5-round(t5))), t5 from PSUM; bf16
  Z3: W[rgb]  [p=(bj,j)-chunk, f=(m,(bi,r))] += rec-slice.T @ (Ai*L*bdiag)
  Z4: PIX     [p=(bi,r)-chunk, f=(m,(bj,c))] = W-slice.T @ blockdiag(H*q)
  (ACT evacuation adds per-channel affine bias, output DMA'd out as bf16)

Soft-quant: with t = coeff/q (+DC offsets) and p = alpha*q^2 large (host
checked p>=30), the reference 5-candidate softmax reduces exactly to
  out/q = round(t-1/2) + sigmoid(2p*(t-1/2 - round(t-1/2)))
Separable folds (rank-1 1/q into A1/A2 cols, rank-1 q into A3/A4) are
host-checked; numpy fallback otherwise.  Inverse side runs bf16.
"""

import math

import numpy as np

# --- fixed problem geometry (hardcoded per harness contract) ---
B_FULL = 64
N_CORES = 8
B_CORE = B_FULL // N_CORES            # 8 images per core
IMG_H = IMG_W = 224
BLK = 8
NBH = IMG_H // BLK                    # 28
NBW = IMG_W // BLK                    # 28
PC = 112                              # uniform chunk size (14 blocks)

MEAN = np.array([0.5071, 0.4867, 0.4408], dtype=np.float64)
STD = np.array([0.2675, 0.2565, 0.2761], dtype=np.float64)
MAGIC = float(np.float32(1.5 * 2.0**23))  # fp32 round-to-nearest trick
WR, WG, WB = 0.299, 0.587, 0.114

_CACHE = {}


def _dct_h():
    i = np.arange(BLK, dtype=np.float64)
    H = np.cos((2.0 * i[:, None] + 1.0) * (i[None, :] * math.pi / (2 * BLK)))
    H = H.astype(np.float32).astype(np.float64)  # match reference's fp32 cast
    n = np.ones(BLK); n[0] = 1.0 / math.sqrt(2.0)
    return H, n


def _color_mats():
    A = np.array([
        [WR, WG, WB],
        [-WR / (2 * (1 - WB)), -WG / (2 * (1 - WB)), (1 - WB) / (2 * (1 - WB))],
        [(1 - WR) / (2 * (1 - WR)), -WG / (2 * (1 - WR)), -WB / (2 * (1 - WR))],
    ])
    Ai = np.array([
        [1.0, 0.0, 2 * (1 - WR)],
        [1.0, -2 * (1 - WB) * WB / WG, -2 * (1 - WR) * WR / WG],
        [1.0, 2 * (1 - WB), 0.0],
    ])
    return A, Ai


def _rank1(M, tol=1e-5):
    """M (8x8, positive) ~= outer(u, v); returns (u, v) or None."""
    if np.any(M <= 0) or not np.all(np.isfinite(M)):
        return None
    u = M[:, 0].copy()
    v = M[0, :] / M[0, 0]
    if np.max(np.abs(np.outer(u, v) - M)) > tol * np.max(np.abs(M)):
        return None
    return u, v


def _host_consts(lum_q, chrom_q, a_lum, a_chrom):
    """Build all host constants, or None if the fast path doesn't apply."""
    ql = lum_q.reshape(BLK, BLK).astype(np.float64)
    qc = chrom_q.reshape(BLK, BLK).astype(np.float64)
    al = a_lum.reshape(BLK, BLK).astype(np.float64)
    ac = a_chrom.reshape(BLK, BLK).astype(np.float64)
    if not (np.allclose(ql, qc, rtol=1e-12) and np.allclose(al, ac, rtol=1e-12)):
        return None
    q, a = ql, al
    r1q = _rank1(q)
    if r1q is None:
        return None
    qu, qv = r1q
    invq = 1.0 / q
    u, v = 1.0 / qu, 1.0 / qv
    p = a * q * q
    if np.max(np.abs(p - p[:, :1])) > 1e-6 * np.max(p) or p.min() < 30.0:
        return None
    if (1024.0 + 5.0) * invq.max() + 1.0 > 124.0:
        return None

    H, n = _dct_h()
    Acol, Ai = _color_mats()
    L = 1.0 / (255.0 * STD)
    Kc = ((128.0 - 0.5 * (Ai[:, 1] + Ai[:, 2])) / 255.0 - MEAN) / STD

    def bdiag(Bm):
        out = np.zeros((112, 112), np.float64)
        for b in range(14):
            out[b * 8:(b + 1) * 8, b * 8:(b + 1) * 8] = Bm
        return out

    B1 = bdiag(H * (n * 0.5 * u)[None, :])          # [r, i]
    B3 = bdiag((H * (n * 0.5 * qu)[None, :]).T)     # [i, r]
    B4 = bdiag((H * (n * 0.5 * qv)[None, :]).T)     # [j, c]
    A1 = B1
    # A3 variants: [outch(rgb), inch(ycc)] scaled by Ai * L[outch]
    A3 = np.stack([Ai[o, c] * L[o] * B3 for o in range(3) for c in range(3)])
    A3 = A3.reshape(3, 3, 112, 112)
    # A2 per-ycc-channel: [114, 112] with two bias rows (spike, -1/2)
    Bm2 = H * (n * 0.5 * v)[None, :]                # [c, j]
    dcq = (-1024.0 * invq[0, 0], 4.0 * invq[0, 0], 4.0 * invq[0, 0])
    A2 = np.zeros((3, 114, 112), np.float64)
    for ch in range(3):
        A2[ch, 0:112] = bdiag(Bm2)
        A2[ch, 112, 0:112:8] = dcq[ch]              # spike row: j==0 cols
        A2[ch, 113, :] = -0.5                       # ones row: uniform shift
    A4 = B4

    s2p = 2.0 * p[:, 0]
    pv = np.zeros((4, 128), np.float64)
    pv[0, 0:112] = np.tile(s2p, 14)                 # partitions (bi,i)
    pv[1, :], pv[2, :], pv[3, :] = Kc[0], Kc[1], Kc[2]

    br = np.zeros((2, 448), np.float64)             # Z2 stationary bias rows
    br[0, 0:448:8] = 1.0                            # ind(i == 0)
    br[1, :] = 1.0

    import ml_dtypes
    return {
        "A1": A1.astype(np.float32),
        "A2": A2.astype(np.float32),
        "A3": A3.astype(ml_dtypes.bfloat16),
        "A4": A4.astype(ml_dtypes.bfloat16),
        "PV": pv.astype(np.float32), "BR": br.astype(np.float32),
    }


def _build_program():
    import concourse.bass as bass
    import concourse.mybir as mybir
    import concourse.tile as tile
    from contextlib import ExitStack

    f32 = mybir.dt.float32
    bf16 = mybir.dt.bfloat16
    AF = mybir.ActivationFunctionType
    OP = mybir.AluOpType

    nc = bass.Bass()
    x_d = nc.dram_tensor("x", [B_CORE, 3, IMG_H, IMG_W], f32, kind="ExternalInput")
    o_d = nc.dram_tensor("out", [B_CORE, 3, IMG_H, IMG_W], bf16, kind="ExternalOutput")
    a1_d = nc.dram_tensor("A1", [112, 112], f32, kind="ExternalInput")
    a2_d = nc.dram_tensor("A2", [3, 114, 112], f32, kind="ExternalInput")
    a3_d = nc.dram_tensor("A3", [3, 3, 112, 112], bf16, kind="ExternalInput")
    a4_d = nc.dram_tensor("A4", [112, 112], bf16, kind="ExternalInput")
    pv_d = nc.dram_tensor("PV", [4, 128], f32, kind="ExternalInput")
    br_d = nc.dram_tensor("BR", [2, 448], f32, kind="ExternalInput")

    with tile.TileContext(nc) as tc, ExitStack() as ctx:
        consts = ctx.enter_context(tc.tile_pool(name="consts", bufs=1))
        xin = ctx.enter_context(tc.tile_pool(name="xin", bufs=12))
        sbw = ctx.enter_context(tc.tile_pool(name="sbw", bufs=4))
        obuf = ctx.enter_context(tc.tile_pool(name="obuf", bufs=4))
        ps = ctx.enter_context(tc.tile_pool(name="ps", bufs=8, space="PSUM"))

        # consts in order of first use: A1/PV/BR/A2 feed the first image's
        # forward half; A3/A4 aren't read until Z3, so they load last
        A1 = consts.tile([112, 112], f32, name="a1", tag="a1")
        nc.sync.dma_start(out=A1, in_=a1_d[0:112, :])
        pvt = []
        for i, nm in enumerate(("s2p", "kcR", "kcG", "kcB")):
            t = consts.tile([128, 1], f32, name="pv_" + nm, tag="pv_" + nm)
            nc.sync.dma_start(out=t, in_=bass.AP(pv_d, i * 128, [[1, 128], [1, 1]]))
            pvt.append(t)
        s2p_t, kc_t = pvt[0], (pvt[1], pvt[2], pvt[3])
        A2 = [consts.tile([114, 112], f32, name=f"a2_{ch}", tag=f"a2_{ch}")
              for ch in range(3)]
        for ch in range(3):
            nc.scalar.dma_start(out=A2[ch], in_=a2_d[ch])
        A3 = [[consts.tile([112, 112], bf16, name=f"a3_{o}{c}", tag=f"a3_{o}{c}")
               for c in range(3)] for o in range(3)]
        for o in range(3):
            for c in range(3):
                eng = nc.sync if (o + c) % 2 == 0 else nc.scalar
                eng.dma_start(out=A3[o][c], in_=a3_d[o, c])
        A4 = consts.tile([112, 112], bf16, name="a4", tag="a4")
        nc.sync.dma_start(out=A4, in_=a4_d[0:112, :])

        mm = nc.tensor.matmul

        def dma_xio(dram, img, ch, sb, to_sbuf):
            # [224,224] DRAM plane <-> [112, 448] tile (col-half = row-chunk)
            off = ((img * 3 + ch) * 224) * 224
            ap = bass.AP(dram, off, [[224, 112], [112 * 224, 2], [1, 224]])
            sb3 = sb.rearrange("p (h w) -> p h w", h=2)
            # alternate the issuing HWDGE queue to split descriptor-gen load
            eng = nc.sync if (img * 3 + ch) % 2 == 0 else nc.scalar
            if to_sbuf:
                eng.dma_start(out=sb3, in_=ap)
            else:
                eng.dma_start(out=ap, in_=sb3)

        KB = 1.0 / (2.0 * (1.0 - WB))
        KR = 1.0 / (2.0 * (1.0 - WR))

        def front_units(img):
            """Yields after each schedulable unit; returns rcm via .value."""
            yts = [sbw.tile([114, 448], f32, name=f"yt{o}", tag=f"yt{o}")
                   for o in range(3)]
            for o in range(3):
                nc.scalar.dma_start(out=yts[o][112:114, :], in_=br_d[0:2, :])
            xt = []
            for ch in range(3):
                x1 = xin.tile([112, 448], f32, name=f"x_{img}_{ch}", tag=f"x{ch}")
                dma_xio(x_d, img, ch, x1, True)
                xt.append(x1)
            yield None

            # Z1 per input channel: VT[c] psum [112, 448]
            vts = []
            for c in range(3):
                v = ps.tile([112, 448], f32, name=f"vt_{img}_{c}", tag="ps")
                for m in range(2):
                    for k in range(2):
                        mm(v[:, 224 * m + 112 * k: 224 * m + 112 * k + 112],
                           xt[c][:, 224 * k + 112 * m: 224 * k + 112 * m + 112],
                           A1, start=True, stop=True)
                vts.append(v)
                yield None

            # fwd color on DVE/ACT
            Rv, Gv, Bv = vts
            Yt, Cbt, Crt = (y[0:112, :] for y in yts)
            t1 = sbw.tile([112, 448], f32, name="t1", tag="t1")
            t2 = sbw.tile([112, 448], f32, name="t2", tag="t2")
            vb = sbw.tile([112, 448], f32, name="vb", tag="vb")
            vr = sbw.tile([112, 448], f32, name="vr", tag="vr")
            nc.scalar.activation(t1, Rv, AF.Identity, bias=0.0, scale=WR)
            nc.vector.scalar_tensor_tensor(t2, Gv, WG, t1, OP.mult, OP.add)
            nc.vector.scalar_tensor_tensor(Yt, Bv, WB, t2, OP.mult, OP.add)
            nc.scalar.activation(vb, Yt, AF.Identity, bias=0.0, scale=KB)
            nc.vector.scalar_tensor_tensor(Cbt, Bv, KB, vb, OP.mult, OP.subtract)
            nc.scalar.activation(vr, Yt, AF.Identity, bias=0.0, scale=KR)
            nc.vector.scalar_tensor_tensor(Crt, Rv, KR, vr, OP.mult, OP.subtract)
            yield None

            # Z2 + quant per channel
            rcm = sbw.tile([112, 1344], bf16, name="rcm", tag="rcm")
            for ch in range(3):
                ct = ps.tile([112, 448], f32, name=f"c_{img}_{ch}", tag="ps")
                for m in range(2):
                    for k in range(2):
                        mm(ct[:, 224 * m + 112 * k: 224 * m + 112 * k + 112],
                           yts[ch][0:114, 224 * k + 112 * m: 224 * k + 112 * m + 112],
                           A2[ch], start=True, stop=True)
                rt = sbw.tile([112, 448], f32, name="rt", tag=f"rt{ch}")
                vv = sbw.tile([112, 448], f32, name="vv", tag=f"vv{ch}")
                sg = sbw.tile([112, 448], f32, name="sg", tag=f"sg{ch}")
                nc.vector.tensor_scalar(rt, ct, MAGIC, MAGIC,
                                        OP.add, OP.subtract)
                nc.vector.tensor_tensor(vv, ct, rt, OP.subtract)
                nc.scalar.activation(sg, vv, AF.Sigmoid, bias=0.0,
                                     scale=s2p_t[0:112, 0:1])
                nc.vector.tensor_tensor(rcm[:, 448 * ch: 448 * ch + 448],
                                        rt, sg, OP.add)
                yield rcm if ch == 2 else None

        def back_units(img, rcm):
            # Z3 (+inv color+L) per rgb out-channel
            rgs = []
            for o in range(3):
                w = ps.tile([112, 448], f32, name=f"w_{img}_{o}", tag="ps")
                for m2 in range(2):
                    for k2 in range(2):
                        for c in range(3):
                            base = 448 * c + 224 * k2 + 112 * m2
                            mm(w[:, 224 * m2 + 112 * k2: 224 * m2 + 112 * k2 + 112],
                               rcm[:, base: base + 112],
                               A3[o][c], start=(c == 0), stop=(c == 2))
                rg = sbw.tile([112, 448], bf16, name="rg", tag=f"rg{o}")
                if o == 0:
                    nc.scalar.activation(rg, w, AF.Identity, bias=0.0, scale=1.0)
                else:
                    nc.vector.tensor_scalar(rg, w, 0.0, None, OP.add)
                rgs.append(rg)
                yield None

            # Z4 + affine evac + store
            for o in range(3):
                pt = ps.tile([112, 448], f32, name=f"p_{img}_{o}", tag="ps")
                for m3 in range(2):
                    for k3 in range(2):
                        mm(pt[:, 224 * m3 + 112 * k3: 224 * m3 + 112 * k3 + 112],
                           rgs[o][:, 224 * k3 + 112 * m3: 224 * k3 + 112 * m3 + 112],
                           A4, start=True, stop=True)
                ot = obuf.tile([112, 448], bf16, name="ot", tag=f"ot{o}")
                nc.scalar.activation(ot, pt, AF.Identity,
                                     bias=kc_t[o][0:112, 0:1], scale=1.0)
                dma_xio(o_d, img, o, ot, False)
                yield None

        # software-pipelined emission at unit granularity: units of img i's
        # front half alternate with units of img i-1's back half
        def drive(front, back):
            r = None
            while front is not None or back is not None:
                if front is not None:
                    try:
                        v = next(front)
                        if v is not None:
                            r = v
                    except StopIteration:
                        front = None
                if back is not None:
                    try:
                        next(back)
                    except StopIteration:
                        back = None
            return r

        prev_rcm = None
        for img in range(B_CORE):
            f = front_units(img)
            b = back_units(img - 1, prev_rcm) if prev_rcm is not None else None
            prev_rcm = drive(f, b)
        drive(None, back_units(B_CORE - 1, prev_rcm))

    # Legalize for walrus codegen: each instruction may carry at most one
    # sync wait (Bacc runs the same passes in its compile()).
    import bass_rust
    bass_rust.move_matmul_waits_to_ldweights(nc.m)
    bass_rust.generate_event_semaphores(nc)
    return nc


def _numpy_reference(input_RGB, lum_qtable, chrom_qtable, alpha_lum, alpha_chrom):
    """fp32-faithful mirror of the JAX reference (same op order/dtypes)."""
    f = np.float32
    NB = NBH * NBW
    x = input_RGB.astype(f) - f(128.0)
    Wr, Wg, Wb = f(WR), f(WG), f(WB)
    r, g, b = x[:, 0], x[:, 1], x[:, 2]
    y = Wr * r + Wg * g + Wb * b
    cb = (b - y) / (2 * (1 - Wb)) + f(0.5)
    cr = (r - y) / (2 * (1 - Wr)) + f(0.5)
    ycc = np.stack((y, cb, cr), axis=1)
    bs = ycc.shape[0]
    blk = ycc.reshape(bs, 3, NBH, BLK, NBW, BLK).transpose(0, 1, 2, 4, 3, 5)
    blk = blk.reshape(bs, 3, NB, BLK, BLK).astype(f)
    i = np.arange(BLK, dtype=np.float64)
    H = np.cos((2.0 * i[:, None] + 1.0) * (i[None, :] * math.pi / (2 * BLK))).astype(f)
    v = np.ones(BLK, dtype=f); v[0] = f(1.0 / math.sqrt(2.0))
    N = (v[:, None] * v[None, :]).astype(f)
    S = f(1.0 / math.sqrt(2.0 * BLK))
    dct = S * N * np.einsum('rk,bcnrs,sm->bcnkm', H, blk, H)
    dct = dct.astype(f)[..., None]

    def soft_quant(inp, qt, al):
        qt = qt.reshape(1, 1, 1, BLK, BLK, 1).astype(f)
        al = al.reshape(1, 1, 1, BLK, BLK, 1).astype(f)
        idx = np.round(inp / qt)
        idx = np.clip(idx - 2, -127.0, 123.0).astype(f)
        idx = idx + np.arange(5, dtype=f)
        iq = idx * qt
        dist = np.square(iq - inp)
        e = (-al * dist).astype(f)
        e = e - e.max(-1, keepdims=True)
        with np.errstate(under='ignore'):
            w = np.exp(e)
        w = w / w.sum(-1, keepdims=True)
        return (w * iq).sum(-1).astype(f)

    rec_l = soft_quant(dct[:, 0:1], lum_qtable, alpha_lum)
    rec_c = soft_quant(dct[:, 1:3], chrom_qtable, alpha_chrom)
    rec = np.concatenate((rec_l, rec_c), axis=1)
    im = S * np.einsum('rk,bcnkm,sm->bcnrs', H, (N * rec).astype(f), H)
    im = im.astype(f).reshape(bs, 3, NBH, NBW, BLK, BLK).transpose(0, 1, 2, 4, 3, 5)
    im = im.reshape(bs, 3, IMG_H, IMG_W)
    yy, cbb, crr = im[:, 0], im[:, 1] - f(0.5), im[:, 2] - f(0.5)
    ro = yy + 2 * (1 - Wr) * crr
    go = yy - 2 * (1 - Wr) * Wr / Wg * crr - 2 * (1 - Wb) * Wb / Wg * cbb
    bo = yy + 2 * (1 - Wb) * cbb
    img = (np.stack((ro, go, bo), axis=1) + f(128.0)) / f(255.0)
    mean = np.array(MEAN, dtype=f).reshape(1, 3, 1, 1)
    std = np.array(STD, dtype=f).reshape(1, 3, 1, 1)
    return ((img - mean) / std).astype(f)


def _get_program():
    if "nc" not in _CACHE:
        _CACHE["nc"] = _build_program()
    return _CACHE["nc"]


def _ensure_ntff_hook():
    """Install the antenv.axon_hooks shim so trace=True can capture NTFF."""
    import sys
    import types
    try:
        import antenv
        if hasattr(antenv, "axon_hooks"):
            return True
        from trn_agent_boot.trn_boot import _ntff_profile_via_ctypes
        hook = _ntff_profile_via_ctypes("/opt/axon/libaxon_pjrt.so")
        if hook is None:
            return False
        mod = types.ModuleType("antenv.axon_hooks")
        mod._hook = hook
        mod.get_axon_ntff_profile_hook = lambda: mod._hook
        mod.set_axon_ntff_profile_hook = lambda h: setattr(mod, "_hook", h)
        sys.modules["antenv.axon_hooks"] = mod
        antenv.axon_hooks = mod
        return True
    except Exception:
        return False


def _run_bass(x, consts, want_trace):
    from concourse import bass_utils

    if want_trace and not _ensure_ntff_hook():
        want_trace = False
    if want_trace:
        # no bucket access in this container; keep artifacts local
        bass_utils.upload_artifacts = lambda tmpdir: str(tmpdir)
    nc = _get_program()
    in_maps = []
    for ci in range(N_CORES):
        in_maps.append({
            "x": np.ascontiguousarray(x[ci * B_CORE:(ci + 1) * B_CORE]),
            "A1": consts["A1"], "A2": consts["A2"],
            "A3": consts["A3"], "A4": consts["A4"],
            "PV": consts["PV"], "BR": consts["BR"],
        })
    res = bass_utils.run_bass_kernel_spmd(
        nc, in_maps, core_ids=list(range(N_CORES)), trace=want_trace)
    out = np.concatenate(
        [np.asarray(r["out"]).astype(np.float32) for r in res.results], axis=0)
    return out, res.exec_time_ns


def kernel(input_RGB, lum_qtable, chrom_qtable, alpha_lum, alpha_chrom,
           _want_trace=False):
    input_RGB = np.ascontiguousarray(np.asarray(input_RGB, dtype=np.float32))
    lum_q = np.asarray(lum_qtable, dtype=np.float32)
    chrom_q = np.asarray(chrom_qtable, dtype=np.float32)
    a_l = np.asarray(alpha_lum, dtype=np.float32)
    a_c = np.asarray(alpha_chrom, dtype=np.float32)
    kernel.last_exec_time_ns = None
    consts = _host_consts(lum_q, chrom_q, a_l, a_c)
    if consts is not None:
        try:
            out, t_ns = _run_bass(input_RGB, consts, _want_trace)
            kernel.last_exec_time_ns = t_ns
            return out
        except Exception:
            import traceback
            traceback.print_exc()
    return _numpy_reference(input_RGB, lum_q, chrom_q, a_l, a_c)


# revision 44
# speedup vs baseline: 83524.3592x; 83524.3592x over previous
"""Trainium2 Bass kernel for the differentiable-JPEG layer.

Zigzag separable-DCT design (per core; data parallel over batch, 8 imgs/core):

Every matmul makes the IMAGE DATA the stationary operand and streams a small
block-diagonal DCT matrix as the moving operand.  Because PE computes
out = lhsT.T @ rhs, each stage flips the partition/free orientation of the
data -- so the blockify / transpose required between the two separable DCT
axes falls out for free and no explicit transpose or gather ever happens.
Both color conversions are folded into the PE stages as per-(outch,inch)
scaled variants of the moving DCT matrix, reusing each stationary data
slice three times.

All row/col spaces are chunked 112+112 (14 blocks of 8), and the two chunks
of every intermediate live side by side in one [112, 448] tile, so each
elementwise op covers a full (img, ch) plane in one instruction.

Per (img, ch) with X = [rows 224 = (bi,r), cols 224 = (bj,c)]:
  Z1: VT[ycc] [p=w-chunk, f=(m,(bi,i))] += X-slice.T @ (colw * blockdiag(H))
  (plain ACT evacuation to SBUF + 2 DMA'd bias K-rows)
  Z2: C[ycc]  [p=(bi,i)-chunk, f=(m,(bj,j))] = Yt-slice.T @ A2[ch-variant]
      (A2 carries two bias rows: DC spike at (i=0,j=0) and uniform -1/2)
  quant: rec = round(t5) + sigmoid(2p*(t5-round(t5))), t5 from PSUM; bf16
  Z3: W[rgb]  [p=(bj,j)-chunk, f=(m,(bi,r))] += rec-slice.T @ (Ai*L*bdiag)
  Z4: PIX     [p=(bi,r)-chunk, f=(m,(bj,c))] = W-slice.T @ blockdiag(H*q)
  (ACT evacuation adds per-channel affine bias, output DMA'd out as bf16)

Soft-quant: with t = coeff/q (+DC offsets) and p = alpha*q^2 large (host
checked p>=30), the reference 5-candidate softmax reduces exactly to
  out/q = round(t-1/2) + sigmoid(2p*(t-1/2 - round(t-1/2)))
Separable folds (rank-1 1/q into A1/A2 cols, rank-1 q into A3/A4) are
host-checked; numpy fallback otherwise.  Inverse side runs bf16.
"""

import math

import numpy as np

# --- fixed problem geometry (hardcoded per harness contract) ---
B_FULL = 64
N_CORES = 8
B_CORE = B_FULL // N_CORES            # 8 images per core
IMG_H = IMG_W = 224
BLK = 8
NBH = IMG_H // BLK                    # 28
NBW = IMG_W // BLK                    # 28
PC = 112                              # uniform chunk size (14 blocks)

MEAN = np.array([0.5071, 0.4867, 0.4408], dtype=np.float64)
STD = np.array([0.2675, 0.2565, 0.2761], dtype=np.float64)
MAGIC = float(np.float32(1.5 * 2.0**23))  # fp32 round-to-nearest trick
WR, WG, WB = 0.299, 0.587, 0.114

_CACHE = {}


def _dct_h():
    i = np.arange(BLK, dtype=np.float64)
    H = np.cos((2.0 * i[:, None] + 1.0) * (i[None, :] * math.pi / (2 * BLK)))
    H = H.astype(np.float32).astype(np.float64)  # match reference's fp32 cast
    n = np.ones(BLK); n[0] = 1.0 / math.sqrt(2.0)
    return H, n


def _color_mats():
    A = np.array([
        [WR, WG, WB],
        [-WR / (2 * (1 - WB)), -WG / (2 * (1 - WB)), (1 - WB) / (2 * (1 - WB))],
        [(1 - WR) / (2 * (1 - WR)), -WG / (2 * (1 - WR)), -WB / (2 * (1 - WR))],
    ])
    Ai = np.array([
        [1.0, 0.0, 2 * (1 - WR)],
        [1.0, -2 * (1 - WB) * WB / WG, -2 * (1 - WR) * WR / WG],
        [1.0, 2 * (1 - WB), 0.0],
    ])
    return A, Ai


def _rank1(M, tol=1e-5):
    """M (8x8, positive) ~= outer(u, v); returns (u, v) or None."""
    if np.any(M <= 0) or not np.all(np.isfinite(M)):
        return None
    u = M[:, 0].copy()
    v = M[0, :] / M[0, 0]
    if np.max(np.abs(np.outer(u, v) - M)) > tol * np.max(np.abs(M)):
        return None
    return u, v


def _host_consts(lum_q, chrom_q, a_lum, a_chrom):
    """Build all host constants, or None if the fast path doesn't apply."""
    ql = lum_q.reshape(BLK, BLK).astype(np.float64)
    qc = chrom_q.reshape(BLK, BLK).astype(np.float64)
    al = a_lum.reshape(BLK, BLK).astype(np.float64)
    ac = a_chrom.reshape(BLK, BLK).astype(np.float64)
    if not (np.allclose(ql, qc, rtol=1e-12) and np.allclose(al, ac, rtol=1e-12)):
        return None
    q, a = ql, al
    r1q = _rank1(q)
    if r1q is None:
        return None
    qu, qv = r1q
    invq = 1.0 / q
    u, v = 1.0 / qu, 1.0 / qv
    p = a * q * q
    if np.max(np.abs(p - p[:, :1])) > 1e-6 * np.max(p) or p.min() < 30.0:
        return None
    if (1024.0 + 5.0) * invq.max() + 1.0 > 124.0:
        return None

    H, n = _dct_h()
    Acol, Ai = _color_mats()
    L = 1.0 / (255.0 * STD)
    Kc = ((128.0 - 0.5 * (Ai[:, 1] + Ai[:, 2])) / 255.0 - MEAN) / STD

    def bdiag(Bm):
        out = np.zeros((112, 112), np.float64)
        for b in range(14):
            out[b * 8:(b + 1) * 8, b * 8:(b + 1) * 8] = Bm
        return out

    B1 = bdiag(H * (n * 0.5 * u)[None, :])          # [r, i]
    B3 = bdiag((H * (n * 0.5 * qu)[None, :]).T)     # [i, r]
    B4 = bdiag((H * (n * 0.5 * qv)[None, :]).T)     # [j, c]
    A1 = B1
    # A3 variants: [outch(rgb), inch(ycc)] scaled by Ai * L[outch]
    A3 = np.stack([Ai[o, c] * L[o] * B3 for o in range(3) for c in range(3)])
    A3 = A3.reshape(3, 3, 112, 112)
    # A2 per-ycc-channel: [114, 112] with two bias rows (spike, -1/2)
    Bm2 = H * (n * 0.5 * v)[None, :]                # [c, j]
    dcq = (-1024.0 * invq[0, 0], 4.0 * invq[0, 0], 4.0 * invq[0, 0])
    A2 = np.zeros((3, 114, 112), np.float64)
    for ch in range(3):
        A2[ch, 0:112] = bdiag(Bm2)
        A2[ch, 112, 0:112:8] = dcq[ch]              # spike row: j==0 cols
        A2[ch, 113, :] = -0.5                       # ones row: uniform shift
    A4 = B4

    s2p = 2.0 * p[:, 0]
    pv = np.zeros((4, 128), np.float64)
    pv[0, 0:112] = np.tile(s2p, 14)                 # partitions (bi,i)
    pv[1, :], pv[2, :], pv[3, :] = Kc[0], Kc[1], Kc[2]

    br = np.zeros((2, 448), np.float64)             # Z2 stationary bias rows
    br[0, 0:448:8] = 1.0                            # ind(i == 0)
    br[1, :] = 1.0

    import ml_dtypes
    return {
        "A1": A1.astype(np.float32),
        "A2": A2.astype(np.float32),
        "A3": A3.astype(ml_dtypes.bfloat16),
        "A4": A4.astype(ml_dtypes.bfloat16),
        "PV": pv.astype(np.float32), "BR": br.astype(np.float32),
    }


def _build_program():
    import concourse.bass as bass
    import concourse.mybir as mybir
    import concourse.tile as tile
    from contextlib import ExitStack

    f32 = mybir.dt.float32
    bf16 = mybir.dt.bfloat16
    AF = mybir.ActivationFunctionType
    OP = mybir.AluOpType

    nc = bass.Bass()
    x_d = nc.dram_tensor("x", [B_CORE, 3, IMG_H, IMG_W], f32, kind="ExternalInput")
    o_d = nc.dram_tensor("out", [B_CORE, 3, IMG_H, IMG_W], bf16, kind="ExternalOutput")
    a1_d = nc.dram_tensor("A1", [112, 112], f32, kind="ExternalInput")
    a2_d = nc.dram_tensor("A2", [3, 114, 112], f32, kind="ExternalInput")
    a3_d = nc.dram_tensor("A3", [3, 3, 112, 112], bf16, kind="ExternalInput")
    a4_d = nc.dram_tensor("A4", [112, 112], bf16, kind="ExternalInput")
    pv_d = nc.dram_tensor("PV", [4, 128], f32, kind="ExternalInput")
    br_d = nc.dram_tensor("BR", [2, 448], f32, kind="ExternalInput")

    with tile.TileContext(nc) as tc, ExitStack() as ctx:
        consts = ctx.enter_context(tc.tile_pool(name="consts", bufs=1))
        xin = ctx.enter_context(tc.tile_pool(name="xin", bufs=9))
        sbw = ctx.enter_context(tc.tile_pool(name="sbw", bufs=3))
        obuf = ctx.enter_context(tc.tile_pool(name="obuf", bufs=3))
        ps = ctx.enter_context(tc.tile_pool(name="ps", bufs=8, space="PSUM"))

        # consts in order of first use: A1/PV/BR/A2 feed the first image's
        # forward half; A3/A4 aren't read until Z3, so they load last
        A1 = consts.tile([112, 112], f32, name="a1", tag="a1")
        nc.sync.dma_start(out=A1, in_=a1_d[0:112, :])
        pvt = []
        for i, nm in enumerate(("s2p", "kcR", "kcG", "kcB")):
            t = consts.tile([128, 1], f32, name="pv_" + nm, tag="pv_" + nm)
            nc.sync.dma_start(out=t, in_=bass.AP(pv_d, i * 128, [[1, 128], [1, 1]]))
            pvt.append(t)
        s2p_t, kc_t = pvt[0], (pvt[1], pvt[2], pvt[3])
        A2 = [consts.tile([114, 112], f32, name=f"a2_{ch}", tag=f"a2_{ch}")
              for ch in range(3)]
        for ch in range(3):
            nc.scalar.dma_start(out=A2[ch], in_=a2_d[ch])
        A3 = [[consts.tile([112, 112], bf16, name=f"a3_{o}{c}", tag=f"a3_{o}{c}")
               for c in range(3)] for o in range(3)]
        for o in range(3):
            for c in range(3):
                eng = nc.sync if (o + c) % 2 == 0 else nc.scalar
                eng.dma_start(out=A3[o][c], in_=a3_d[o, c])
        A4 = consts.tile([112, 112], bf16, name="a4", tag="a4")
        nc.sync.dma_start(out=A4, in_=a4_d[0:112, :])

        mm = nc.tensor.matmul

        def dma_xio(dram, img, ch, sb, to_sbuf):
            # [224,224] DRAM plane <-> [112, 448] tile (col-half = row-chunk)
            off = ((img * 3 + ch) * 224) * 224
            ap = bass.AP(dram, off, [[224, 112], [112 * 224, 2], [1, 224]])
            sb3 = sb.rearrange("p (h w) -> p h w", h=2)
            # alternate the issuing HWDGE queue to split descriptor-gen load
            eng = nc.sync if (img * 3 + ch) % 2 == 0 else nc.scalar
            if to_sbuf:
                eng.dma_start(out=sb3, in_=ap)
            else:
                eng.dma_start(out=ap, in_=sb3)

        KB = 1.0 / (2.0 * (1.0 - WB))
        KR = 1.0 / (2.0 * (1.0 - WR))

        def front_units(img):
            """Yields after each schedulable unit; returns rcm via .value."""
            yts = [sbw.tile([114, 448], f32, name=f"yt{o}", tag=f"yt{o}")
                   for o in range(3)]
            for o in range(3):
                nc.scalar.dma_start(out=yts[o][112:114, :], in_=br_d[0:2, :])
            xt = []
            for ch in range(3):
                x1 = xin.tile([112, 448], f32, name=f"x_{img}_{ch}", tag=f"x{ch}")
                dma_xio(x_d, img, ch, x1, True)
                xt.append(x1)
            yield None

            # Z1 per input channel: VT[c] psum [112, 448]
            vts = []
            for c in range(3):
                v = ps.tile([112, 448], f32, name=f"vt_{img}_{c}", tag="ps")
                for m in range(2):
                    for k in range(2):
                        mm(v[:, 224 * m + 112 * k: 224 * m + 112 * k + 112],
                           xt[c][:, 224 * k + 112 * m: 224 * k + 112 * m + 112],
                           A1, start=True, stop=True)
                vts.append(v)
                yield None

            # fwd color on DVE/ACT
            Rv, Gv, Bv = vts
            Yt, Cbt, Crt = (y[0:112, :] for y in yts)
            t1 = sbw.tile([112, 448], f32, name="t1", tag="t1")
            t2 = sbw.tile([112, 448], f32, name="t2", tag="t2")
            vb = sbw.tile([112, 448], f32, name="vb", tag="vb")
            vr = sbw.tile([112, 448], f32, name="vr", tag="vr")
            nc.scalar.activation(t1, Rv, AF.Identity, bias=0.0, scale=WR)
            nc.vector.scalar_tensor_tensor(t2, Gv, WG, t1, OP.mult, OP.add)
            nc.vector.scalar_tensor_tensor(Yt, Bv, WB, t2, OP.mult, OP.add)
            nc.scalar.activation(vb, Yt, AF.Identity, bias=0.0, scale=KB)
            nc.vector.scalar_tensor_tensor(Cbt, Bv, KB, vb, OP.mult, OP.subtract)
            nc.scalar.activation(vr, Yt, AF.Identity, bias=0.0, scale=KR)
            nc.vector.scalar_tensor_tensor(Crt, Rv, KR, vr, OP.mult, OP.subtract)
            yield None

            # Z2 + quant per channel
            rcm = sbw.tile([112, 1344], bf16, name="rcm", tag="rcm")
            for ch in range(3):
                ct = ps.tile([112, 448], f32, name=f"c_{img}_{ch}", tag="ps")
                for m in range(2):
                    for k in range(2):
                        mm(ct[:, 224 * m + 112 * k: 224 * m + 112 * k + 112],
                           yts[ch][0:114, 224 * k + 112 * m: 224 * k + 112 * m + 112],
                           A2[ch], start=True, stop=True)
                rt = sbw.tile([112, 448], f32, name="rt", tag=f"rt{ch}")
                vv = sbw.tile([112, 448], f32, name="vv", tag=f"vv{ch}")
                sg = sbw.tile([112, 448], f32, name="sg", tag=f"sg{ch}")
                nc.vector.tensor_scalar(rt, ct, MAGIC, MAGIC,
                                        OP.add, OP.subtract)
                nc.vector.tensor_tensor(vv, ct, rt, OP.subtract)
                nc.scalar.activation(sg, vv, AF.Sigmoid, bias=0.0,
                                     scale=s2p_t[0:112, 0:1])
                nc.vector.tensor_tensor(rcm[:, 448 * ch: 448 * ch + 448],
                                        rt, sg, OP.add)
                yield rcm if ch == 2 else None

        def back_units(img, rcm):
            # Z3 (+inv color+L) per rgb out-channel
            rgs = []
            for o in range(3):
                w = ps.tile([112, 448], f32, name=f"w_{img}_{o}", tag="ps")
                for m2 in range(2):
                    for k2 in range(2):
                        for c in range(3):
                            base = 448 * c + 224 * k2 + 112 * m2
                            mm(w[:, 224 * m2 + 112 * k2: 224 * m2 + 112 * k2 + 112],
                               rcm[:, base: base + 112],
                               A3[o][c], start=(c == 0), stop=(c == 2))
                rg = sbw.tile([112, 448], bf16, name="rg", tag=f"rg{o}")
                if o == 0:
                    nc.scalar.activation(rg, w, AF.Identity, bias=0.0, scale=1.0)
                else:
                    nc.vector.tensor_scalar(rg, w, 0.0, None, OP.add)
                rgs.append(rg)
                yield None

            # Z4 + affine evac + store
            for o in range(3):
                pt = ps.tile([112, 448], f32, name=f"p_{img}_{o}", tag="ps")
                for m3 in range(2):
                    for k3 in range(2):
                        mm(pt[:, 224 * m3 + 112 * k3: 224 * m3 + 112 * k3 + 112],
                           rgs[o][:, 224 * k3 + 112 * m3: 224 * k3 + 112 * m3 + 112],
                           A4, start=True, stop=True)
                ot = obuf.tile([112, 448], bf16, name="ot", tag=f"ot{o}")
                nc.scalar.activation(ot, pt, AF.Identity,
                                     bias=kc_t[o][0:112, 0:1], scale=1.0)
                dma_xio(o_d, img, o, ot, False)
                yield None

        # software-pipelined emission at unit granularity: units of img i's
        # front half alternate with units of img i-1's back half
        def drive(front, back):
            r = None
            while front is not None or back is not None:
                if front is not None:
                    try:
                        v = next(front)
                        if v is not None:
                            r = v
                    except StopIteration:
                        front = None
                if back is not None:
                    try:
                        next(back)
                    except StopIteration:
                        back = None
            return r

        prev_rcm = None
        for img in range(B_CORE):
            f = front_units(img)
            b = back_units(img - 1, prev_rcm) if prev_rcm is not None else None
            prev_rcm = drive(f, b)
        drive(None, back_units(B_CORE - 1, prev_rcm))

    # Legalize for walrus codegen: each instruction may carry at most one
    # sync wait (Bacc runs the same passes in its compile()).
    import bass_rust
    bass_rust.move_matmul_waits_to_ldweights(nc.m)
    bass_rust.generate_event_semaphores(nc)
    return nc


def _numpy_reference(input_RGB, lum_qtable, chrom_qtable, alpha_lum, alpha_chrom):
    """fp32-faithful mirror of the JAX reference (same op order/dtypes)."""
    f = np.float32
    NB = NBH * NBW
    x = input_RGB.astype(f) - f(128.0)
    Wr, Wg, Wb = f(WR), f(WG), f(WB)
    r, g, b = x[:, 0], x[:, 1], x[:, 2]
    y = Wr * r + Wg * g + Wb * b
    cb = (b - y) / (2 * (1 - Wb)) + f(0.5)
    cr = (r - y) / (2 * (1 - Wr)) + f(0.5)
    ycc = np.stack((y, cb, cr), axis=1)
    bs = ycc.shape[0]
    blk = ycc.reshape(bs, 3, NBH, BLK, NBW, BLK).transpose(0, 1, 2, 4, 3, 5)
    blk = blk.reshape(bs, 3, NB, BLK, BLK).astype(f)
    i = np.arange(BLK, dtype=np.float64)
    H = np.cos((2.0 * i[:, None] + 1.0) * (i[None, :] * math.pi / (2 * BLK))).astype(f)
    v = np.ones(BLK, dtype=f); v[0] = f(1.0 / math.sqrt(2.0))
    N = (v[:, None] * v[None, :]).astype(f)
    S = f(1.0 / math.sqrt(2.0 * BLK))
    dct = S * N * np.einsum('rk,bcnrs,sm->bcnkm', H, blk, H)
    dct = dct.astype(f)[..., None]

    def soft_quant(inp, qt, al):
        qt = qt.reshape(1, 1, 1, BLK, BLK, 1).astype(f)
        al = al.reshape(1, 1, 1, BLK, BLK, 1).astype(f)
        idx = np.round(inp / qt)
        idx = np.clip(idx - 2, -127.0, 123.0).astype(f)
        idx = idx + np.arange(5, dtype=f)
        iq = idx * qt
        dist = np.square(iq - inp)
        e = (-al * dist).astype(f)
        e = e - e.max(-1, keepdims=True)
        with np.errstate(under='ignore'):
            w = np.exp(e)
        w = w / w.sum(-1, keepdims=True)
        return (w * iq).sum(-1).astype(f)

    rec_l = soft_quant(dct[:, 0:1], lum_qtable, alpha_lum)
    rec_c = soft_quant(dct[:, 1:3], chrom_qtable, alpha_chrom)
    rec = np.concatenate((rec_l, rec_c), axis=1)
    im = S * np.einsum('rk,bcnkm,sm->bcnrs', H, (N * rec).astype(f), H)
    im = im.astype(f).reshape(bs, 3, NBH, NBW, BLK, BLK).transpose(0, 1, 2, 4, 3, 5)
    im = im.reshape(bs, 3, IMG_H, IMG_W)
    yy, cbb, crr = im[:, 0], im[:, 1] - f(0.5), im[:, 2] - f(0.5)
    ro = yy + 2 * (1 - Wr) * crr
    go = yy - 2 * (1 - Wr) * Wr / Wg * crr - 2 * (1 - Wb) * Wb / Wg * cbb
    bo = yy + 2 * (1 - Wb) * cbb
    img = (np.stack((ro, go, bo), axis=1) + f(128.0)) / f(255.0)
    mean = np.array(MEAN, dtype=f).reshape(1, 3, 1, 1)
    std = np.array(STD, dtype=f).reshape(1, 3, 1, 1)
    return ((img - mean) / std).astype(f)


def _get_program():
    if "nc" not in _CACHE:
        _CACHE["nc"] = _build_program()
    return _CACHE["nc"]


def _ensure_ntff_hook():
    """Install the antenv.axon_hooks shim so trace=True can capture NTFF."""
    import sys
    import types
    try:
        import antenv
        if hasattr(antenv, "axon_hooks"):
            return True
        from trn_agent_boot.trn_boot import _ntff_profile_via_ctypes
        hook = _ntff_profile_via_ctypes("/opt/axon/libaxon_pjrt.so")
        if hook is None:
            return False
        mod = types.ModuleType("antenv.axon_hooks")
        mod._hook = hook
        mod.get_axon_ntff_profile_hook = lambda: mod._hook
        mod.set_axon_ntff_profile_hook = lambda h: setattr(mod, "_hook", h)
        sys.modules["antenv.axon_hooks"] = mod
        antenv.axon_hooks = mod
        return True
    except Exception:
        return False


def _run_bass(x, consts, want_trace):
    from concourse import bass_utils

    if want_trace and not _ensure_ntff_hook():
        want_trace = False
    if want_trace:
        # no bucket access in this container; keep artifacts local
        bass_utils.upload_artifacts = lambda tmpdir: str(tmpdir)
    nc = _get_program()
    in_maps = []
    for ci in range(N_CORES):
        in_maps.append({
            "x": np.ascontiguousarray(x[ci * B_CORE:(ci + 1) * B_CORE]),
            "A1": consts["A1"], "A2": consts["A2"],
            "A3": consts["A3"], "A4": consts["A4"],
            "PV": consts["PV"], "BR": consts["BR"],
        })
    res = bass_utils.run_bass_kernel_spmd(
        nc, in_maps, core_ids=list(range(N_CORES)), trace=want_trace)
    out = np.concatenate(
        [np.asarray(r["out"]).astype(np.float32) for r in res.results], axis=0)
    return out, res.exec_time_ns


def kernel(input_RGB, lum_qtable, chrom_qtable, alpha_lum, alpha_chrom,
           _want_trace=False):
    input_RGB = np.ascontiguousarray(np.asarray(input_RGB, dtype=np.float32))
    lum_q = np.asarray(lum_qtable, dtype=np.float32)
    chrom_q = np.asarray(chrom_qtable, dtype=np.float32)
    a_l = np.asarray(alpha_lum, dtype=np.float32)
    a_c = np.asarray(alpha_chrom, dtype=np.float32)
    kernel.last_exec_time_ns = None
    consts = _host_consts(lum_q, chrom_q, a_l, a_c)
    if consts is not None:
        try:
            out, t_ns = _run_bass(input_RGB, consts, _want_trace)
            kernel.last_exec_time_ns = t_ns
            return out
        except Exception:
            import traceback
            traceback.print_exc()
    return _numpy_reference(input_RGB, lum_q, chrom_q, a_l, a_c)
